# revision 32
# baseline (speedup 1.0000x reference)
"""BinaryLinear kernel for Trainium2 (8 NeuronCores, SPMD).

Computes  out = sign(x) @ sign(W)^T * alpha  for
x: [8192, 2048] f32, W: [2048, 2048] f32, alpha: [1] f32.

Strategy: data-parallel over tokens (8 shards of 1024). Every core
reads W^T slices {0,1,2,3} as f32 locally (identical on all cores, so
the program stays SPMD-uniform) plus its OWN 256-col slice, which it
signs to fp8 and contributes to an HBM AllGather; slots {4..7} of the
AllGather output supply the remaining columns. Per-core HBM traffic
~21 MB vs 32 MB for full W replication.

Numerics: x tiles are signed to +-0.5 in one DVE op ((x>0) - 0.5) or
+-1 via ACT sign (late tiles, to unblock the DVE drain queue); W to
+-1 via ACT sign. fp8(E4M3) holds all exactly, PSUM sums are exact,
and each drain scales by 2*alpha or alpha per the tile's encoding.
Output is f16 (integers <= 2048 exact), converted to f32 on host.

DMA: three rings (scalar/Activation, sync/SP, gpsimd/SWDGE) balanced
by bytes, all major transfers with 4-8 KB per-partition runs. W01 is
split across scalar+sync so it lands early (it gates the first
matmul); w23 rides the gpsimd ring; the AllGather slot loads land in a
slot-major SBUF tile (contiguous 4 KB runs) consumed by FD-256
dual-accumulation-group DoubleRow units.
"""

import numpy as np

import concourse.bass as bass
import concourse.tile as tile
from concourse import bacc, mybir
from concourse.bass_utils import run_bass_kernel_spmd

N_CORES = 8
NTOK = 8192
INF = 2048
OUTF = 2048
TPC = NTOK // N_CORES  # tokens per core (1024)
P = 128
KT = INF // P  # 16 contraction tiles
MT = TPC // P  # 8 token tiles per core
SL = OUTF // N_CORES  # 256 out_features per W slice
FD = 512  # PSUM bank free dim

F32 = mybir.dt.float32
F16 = mybir.dt.float16
FP8 = mybir.dt.float8e4

# x tiles signed on ACT (+-1) instead of DVE (+-0.5). Empty for now: all
# x tiles go through the one-op DVE sign.
ACT_X_TILES = ()

# Static PE emission order over fine-grained (slice, m) units. Local
# slices 0-3 stream from f32 one slice at a time while x streams one
# m-tile at a time, so local units run in anti-diagonal (s+m) order to
# match joint arrival. AllGather slices 4-7 interleave from the middle
# (the AG completes mid-flight) with their m7 units last. Each unit is
# one FD-256 DoubleRow accumulation group in its own PSUM bank; 8
# consecutive units share one 4KB-run out DMA.
UNIT_ORDER = (
    [(0, 0), (0, 1), (1, 0), (1, 1), (0, 2), (1, 2), (0, 3), (1, 3)]
    + [(2, 0), (2, 1), (2, 2), (2, 3), (0, 4), (1, 4), (2, 4), (0, 5), (1, 5), (2, 5)]
    + [(s, m) for m in range(6) for s in range(4, 8)]
    + [(3, 0), (3, 1), (3, 2), (3, 3), (3, 4), (3, 5)]
    + [(0, 6), (1, 6), (2, 6), (3, 6), (4, 6), (5, 6), (6, 6), (7, 6)]
    + [(0, 7), (1, 7), (2, 7), (3, 7), (4, 7), (5, 7), (6, 7), (7, 7)]
)
assert len(UNIT_ORDER) == 64 and len(set(UNIT_ORDER)) == 64

_compiled = None
LAST_RESULT = None  # BassKernelResults of the most recent run (for profiling)


def _build():
    nc = bacc.Bacc(
        "TRN2",
        target_bir_lowering=False,
        debug=False,
        num_devices=N_CORES,
    )
    xt = nc.dram_tensor("xt", [MT * P * KT * P], F32, kind="ExternalInput").ap()
    # local W slices 0..3, each 2 chunks of [128, 8, 256] (8KB runs)
    wloc = nc.dram_tensor("wloc", [4 * P * KT * SL], F32, kind="ExternalInput").ap()
    # my slice, 2 chunks [128,8,256]
    wsl = nc.dram_tensor("wsl", [P * KT * SL], F32, kind="ExternalInput").ap()
    al = nc.dram_tensor("alpha", [P, 2], F32, kind="ExternalInput").ap()
    wsg_in = nc.dram_tensor("wsg_in", [P * KT * SL], FP8, kind="Internal")
    wsg_out = nc.dram_tensor(
        "wsg_out", [N_CORES * P * KT * SL], FP8, kind="Internal", addr_space="Shared"
    )
    out = nc.dram_tensor(
        "out", [8, P, 8 * SL], F16, kind="ExternalOutput"
    ).ap()

    with tile.TileContext(nc) as tc:
        with (
            tc.tile_pool(name="res", bufs=1) as res,
            tc.tile_pool(name="wload", bufs=4) as wload,
            tc.tile_pool(name="wsload", bufs=2) as wsload,
            tc.tile_pool(name="xload", bufs=4) as xload,
            tc.tile_pool(name="psum", bufs=8, space="PSUM") as ppool,
            tc.tile_pool(name="outp", bufs=3) as outp,
        ):
            bx = res.tile([P, KT, TPC], FP8)  # x signs, 16 KB/part
            bwl = res.tile([P, KT, 2 * FD], FP8)  # slices 0-3, 16 KB/part
            # AG slots 4-7, slot-major so loads are contiguous 4KB runs
            bwr = res.tile([P, 4, KT, SL], FP8)  # 16 KB/part
            bsl = res.tile([P, KT, SL], FP8)  # my slice fp8, 4 KB/part
            alpha_t = res.tile([P, 2], F32)  # [2*alpha, alpha] from host

            nc.scalar.dma_start(alpha_t[:], al)

            # emission helpers ------------------------------------------------
            def w_chunk(ring, s, half):
                # half-slice chunk: k-tiles [half*8, half*8+8) of slice s
                wf = wload.tile([P, 8, SL], F32, name="wf", tag="wf")
                base = (2 * s + half) * P * 8 * SL
                src = wloc[base : base + P * 8 * SL].rearrange("(p f) -> p f", p=P)
                ring.dma_start(wf[:].rearrange("p a b -> p (a b)"), src)
                nc.scalar.sign(
                    bwl[:, half * 8 : (half + 1) * 8, s * SL : (s + 1) * SL], wf[:]
                )

            def x_chunk(ring, m):
                xf = xload.tile([P, KT, P], F32, name="xf", tag="xf")
                src = xt[m * P * KT * P : (m + 1) * P * KT * P].rearrange(
                    "(p f) -> p f", p=P
                )
                ring.dma_start(xf[:].rearrange("p a b -> p (a b)"), src)
                if m in ACT_X_TILES:
                    nc.scalar.sign(bx[:, :, m * P : (m + 1) * P], xf[:])
                else:
                    nc.vector.tensor_scalar(
                        bx[:, :, m * P : (m + 1) * P], xf[:], 0.0, 0.5,
                        op0=mybir.AluOpType.is_gt, op1=mybir.AluOpType.subtract,
                    )

            def wsl_chunk(ring, i):
                # my slice, front-loaded on the HWDGE lanes; signed to +-0.5
                # on the gpsimd engine so ACT/DVE stay free (AG drains x4a).
                wsf = wsload.tile([P, 8, SL], F32, name="wsf", tag="wsf")
                src = wsl[i * P * 8 * SL : (i + 1) * P * 8 * SL].rearrange(
                    "(p f) -> p f", p=P
                )
                ring.dma_start(wsf[:].rearrange("p a b -> p (a b)"), src)
                nc.gpsimd.tensor_scalar(
                    bsl[:, i * 8 : (i + 1) * 8, :], wsf[:], 0.0, 0.5,
                    op0=mybir.AluOpType.is_gt, op1=mybir.AluOpType.subtract,
                )

            # ring schedules --------------------------------------------------
            # scalar: alpha wsl-a w0a x0 w1a x2 w2a x4 w3a x6
            # sync:   wsl-b w0b x1 w1b x3 w2b x5 w3b x7
            # gpsimd ring: bounce, agloads, outs; engine: wsl signs + the AG.
            wsl_chunk(nc.scalar, 0)
            wsl_chunk(nc.sync, 1)
            nc.gpsimd.dma_start(
                wsg_in.ap().rearrange("(p f) -> p f", p=P),
                bsl[:].rearrange("p a b -> p (a b)"),
            )
            w_chunk(nc.scalar, 0, 0)
            w_chunk(nc.sync, 0, 1)
            x_chunk(nc.scalar, 0)
            x_chunk(nc.sync, 1)
            w_chunk(nc.scalar, 1, 0)
            w_chunk(nc.sync, 1, 1)
            x_chunk(nc.scalar, 2)
            x_chunk(nc.sync, 3)
            w_chunk(nc.scalar, 2, 0)
            w_chunk(nc.sync, 2, 1)
            x_chunk(nc.scalar, 4)
            x_chunk(nc.sync, 5)
            w_chunk(nc.scalar, 3, 0)
            w_chunk(nc.sync, 3, 1)
            x_chunk(nc.scalar, 6)
            x_chunk(nc.sync, 7)

            nc.gpsimd.collective_compute(
                "AllGather",
                mybir.AluOpType.bypass,
                replica_groups=[list(range(N_CORES))],
                ins=[wsg_in.ap()],
                outs=[wsg_out.ap()],
            )
            # AG slot loads (gpsimd ring tail, gated on the AG): slots 4..7
            SLB = P * KT * SL
            for s in range(4, 8):
                src = wsg_out.ap()[s * SLB : (s + 1) * SLB].rearrange(
                    "(p f) -> p f", p=P
                )
                nc.gpsimd.dma_start(
                    bwr[:, s - 4, :, :].rearrange("p a b -> p (a b)"), src
                )

            # -- PE units ----------------------------------------------------
            # one FD-256 DoubleRow accumulation group per unit, in its own
            # PSUM bank (allocated [P, FD] so banks are never shared).
            ob8 = None
            for ui, (s, m) in enumerate(UNIT_ORDER):
                ps = ppool.tile([P, FD], F32, name="ps", tag="ps")
                if s < 4:
                    rhs = lambda kc: bwl[:, 2 * kc : 2 * kc + 2, s * SL : (s + 1) * SL]
                else:
                    rhs = lambda kc: bwr[:, s - 4, 2 * kc : 2 * kc + 2, :]
                for kc in range(KT // 2):
                    nc.tensor.matmul(
                        ps[:, 0:SL],
                        bx[:, 2 * kc : 2 * kc + 2, m * P : (m + 1) * P],
                        rhs(kc),
                        start=(kc == 0),
                        stop=(kc == KT // 2 - 1),
                        perf_mode=mybir.MatmulPerfMode.DoubleRow,
                    )
                if ui % 8 == 0:
                    ob8 = outp.tile([P, 8, SL], F16, name="ob", tag="ob")
                # col0 = 2*alpha (x +-0.5, W +-1); col1 = 4*alpha (AG slices:
                # both operands +-0.5)
                acol = 1 if s >= 4 else 0
                nc.vector.tensor_scalar_mul(
                    ob8[:, ui % 8, :], ps[:, 0:SL], alpha_t[:, acol : acol + 1]
                )
                if ui % 8 == 7:
                    nc.gpsimd.dma_start(
                        out[ui // 8], ob8[:].rearrange("p a b -> p (a b)")
                    )

    nc.compile()
    return nc


def _pack_common(weight):
    WT4 = np.ascontiguousarray(weight.T).reshape(KT, P, OUTF)

    def slice_chunks(s):
        cols = slice(s * SL, (s + 1) * SL)
        return [
            WT4[i * 8 : (i + 1) * 8, :, cols].transpose(1, 0, 2).ravel()
            for i in range(2)
        ]

    wloc = np.ascontiguousarray(
        np.concatenate([c for s in range(4) for c in slice_chunks(s)])
    )
    wsls = [
        np.ascontiguousarray(np.concatenate(slice_chunks(c)))
        for c in range(N_CORES)
    ]
    return wloc, wsls


def _pack_x_shard(xs):
    xT4 = np.ascontiguousarray(xs.T).reshape(KT, P, TPC)
    return np.ascontiguousarray(
        np.concatenate(
            [xT4[:, :, m * P : (m + 1) * P].transpose(1, 0, 2).ravel() for m in range(MT)]
        )
    )


def kernel(x, weight, alpha):
    global _compiled, LAST_RESULT
    if _compiled is None:
        _compiled = _build()
    nc = _compiled

    x = np.asarray(x, dtype=np.float32)
    weight = np.asarray(weight, dtype=np.float32)
    alpha = np.asarray(alpha, dtype=np.float32)

    wloc, wsls = _pack_common(weight)
    a = float(alpha.reshape(-1)[0])
    alv = np.empty((P, 2), dtype=np.float32)
    alv[:, 0] = 2.0 * a
    alv[:, 1] = 4.0 * a
    in_maps = []
    for c in range(N_CORES):
        xs = _pack_x_shard(x[c * TPC : (c + 1) * TPC, :])
        in_maps.append({"xt": xs, "wloc": wloc, "wsl": wsls[c], "alpha": alv})

    LAST_RESULT = run_bass_kernel_spmd(nc, in_maps, list(range(N_CORES)))
    full = np.empty((NTOK, OUTF), dtype=np.float32)
    for c in range(N_CORES):
        o = LAST_RESULT.results[c]["out"].astype(np.float32)  # [8, P, 8*SL]
        o = o.reshape(8, P, 8, SL)
        for ui, (s, m) in enumerate(UNIT_ORDER):
            rows = slice(c * TPC + m * P, c * TPC + (m + 1) * P)
            cols = slice(s * SL, (s + 1) * SL)
            full[rows, cols] = o[ui // 8, :, ui % 8, :]
    return full


# revision 35
# speedup vs baseline: 1.2586x; 1.2586x over previous
"""BinaryLinear kernel for Trainium2 (8 NeuronCores, SPMD).

Computes  out = sign(x) @ sign(W)^T * alpha  for
x: [8192, 2048] f32, W: [2048, 2048] f32, alpha: [1] f32.

Strategy: data-parallel over tokens (8 shards of 1024). Every core
reads W^T slices {0,1,2,3} as f32 locally (identical on all cores, so
the program stays SPMD-uniform) plus its OWN 256-col slice, which it
signs to fp8 and contributes to an HBM AllGather; slots {4..7} of the
AllGather output supply the remaining columns. Per-core HBM traffic
~21 MB vs 32 MB for full W replication.

Numerics: x tiles are signed to +-0.5 in one DVE op ((x>0) - 0.5) or
+-1 via ACT sign (late tiles, to unblock the DVE drain queue); W to
+-1 via ACT sign. fp8(E4M3) holds all exactly, PSUM sums are exact,
and each drain scales by 2*alpha or alpha per the tile's encoding.
Output is f16 (integers <= 2048 exact), converted to f32 on host.

DMA: three rings (scalar/Activation, sync/SP, gpsimd/SWDGE) balanced
by bytes, all major transfers with 4-8 KB per-partition runs. W01 is
split across scalar+sync so it lands early (it gates the first
matmul); w23 rides the gpsimd ring; the AllGather slot loads land in a
slot-major SBUF tile (contiguous 4 KB runs) consumed by FD-256
dual-accumulation-group DoubleRow units.
"""

import numpy as np

import concourse.bass as bass
import concourse.tile as tile
from concourse import bacc, mybir
from concourse.bass_utils import run_bass_kernel_spmd

N_CORES = 8
NTOK = 8192
INF = 2048
OUTF = 2048
TPC = NTOK // N_CORES  # tokens per core (1024)
P = 128
KT = INF // P  # 16 contraction tiles
MT = TPC // P  # 8 token tiles per core
SL = OUTF // N_CORES  # 256 out_features per W slice
FD = 512  # PSUM bank free dim

F32 = mybir.dt.float32
F16 = mybir.dt.float16
FP8 = mybir.dt.float8e4

# x tiles signed on ACT (+-1) instead of DVE (+-0.5). Empty for now: all
# x tiles go through the one-op DVE sign.
ACT_X_TILES = ()

# Static PE emission order over fine-grained (slice, m) units, sorted
# by predicted data readiness. Slices 0-3 come from local f32 (w01 then
# w23), slices 4-7 from the AllGather. Each unit is one FD-256
# DoubleRow accumulation group in its own PSUM bank; 8 consecutive
# units share one 4KB-run out DMA.
UNIT_ORDER = (
    [(0, 0), (1, 0), (0, 1), (1, 1), (0, 2), (1, 2), (0, 3), (1, 3)]
    + [(2, 0), (3, 0), (0, 4), (1, 4), (2, 1), (3, 1), (0, 5), (1, 5)]
    + [(2, 2), (3, 2), (0, 6), (1, 6), (2, 3), (3, 3), (0, 7), (1, 7)]
    + [(2, 4), (3, 4), (2, 5), (3, 5), (2, 6), (3, 6), (2, 7), (3, 7)]
    + [(4, 0), (5, 0), (4, 1), (5, 1), (4, 2), (5, 2), (4, 3), (5, 3)]
    + [(4, 4), (5, 4), (4, 5), (5, 5), (4, 6), (5, 6), (4, 7), (5, 7)]
    + [(6, 0), (7, 0), (6, 1), (7, 1), (6, 2), (7, 2), (6, 3), (7, 3)]
    + [(6, 4), (7, 4), (6, 5), (7, 5), (6, 6), (7, 6), (6, 7), (7, 7)]
)

_compiled = None
LAST_RESULT = None  # BassKernelResults of the most recent run (for profiling)


def _build():
    nc = bacc.Bacc(
        "TRN2",
        target_bir_lowering=False,
        debug=False,
        num_devices=N_CORES,
    )
    xt = nc.dram_tensor("xt", [MT * P * KT * P], F32, kind="ExternalInput").ap()
    # w01: pair {0,1} in 4 k-quad chunks [128,4,512]; w23 likewise
    w01 = nc.dram_tensor("w01", [KT * P * FD], F32, kind="ExternalInput").ap()
    w23 = nc.dram_tensor("w23", [KT * P * FD], F32, kind="ExternalInput").ap()
    # my slice, 2 chunks [128,8,256]
    wsl = nc.dram_tensor("wsl", [P * KT * SL], F32, kind="ExternalInput").ap()
    al = nc.dram_tensor("alpha", [P, 2], F32, kind="ExternalInput").ap()
    wsg_in = nc.dram_tensor("wsg_in", [P * KT * SL], FP8, kind="Internal")
    wsg_out = nc.dram_tensor(
        "wsg_out", [N_CORES * P * KT * SL], FP8, kind="Internal", addr_space="Shared"
    )
    out = nc.dram_tensor(
        "out", [8, P, 8 * SL], F16, kind="ExternalOutput"
    ).ap()

    with tile.TileContext(nc) as tc:
        with (
            tc.tile_pool(name="res", bufs=1) as res,
            tc.tile_pool(name="wload", bufs=6) as wload,
            tc.tile_pool(name="wsload", bufs=2) as wsload,
            tc.tile_pool(name="xload", bufs=6) as xload,
            tc.tile_pool(name="psum", bufs=8, space="PSUM") as ppool,
            tc.tile_pool(name="outp", bufs=4) as outp,
        ):
            bx = res.tile([P, KT, TPC], FP8)  # x signs, 16 KB/part
            bwl = res.tile([P, KT, 2 * FD], FP8)  # slices 0-3, 16 KB/part
            # AG slots 4-7, slot-major so loads are contiguous 4KB runs
            bwr = res.tile([P, 4, KT, SL], FP8)  # 16 KB/part
            bsl = res.tile([P, KT, SL], FP8)  # my slice fp8, 4 KB/part
            alpha_t = res.tile([P, 2], F32)  # [2*alpha, alpha] from host

            nc.scalar.dma_start(alpha_t[:], al)

            # emission helpers ------------------------------------------------
            def w01_chunk(ring, kq):
                wf = wload.tile([P, 4, FD], F32, name="wf", tag="wf")
                src = w01[kq * P * 4 * FD : (kq + 1) * P * 4 * FD].rearrange(
                    "(p f) -> p f", p=P
                )
                ring.dma_start(wf[:].rearrange("p a b -> p (a b)"), src)
                nc.scalar.sign(bwl[:, kq * 4 : (kq + 1) * 4, 0:FD], wf[:])

            def w23_chunk(ring, kq):
                wf = wload.tile([P, 4, FD], F32, name="wf", tag="wf")
                src = w23[kq * P * 4 * FD : (kq + 1) * P * 4 * FD].rearrange(
                    "(p f) -> p f", p=P
                )
                ring.dma_start(wf[:].rearrange("p a b -> p (a b)"), src)
                nc.scalar.sign(bwl[:, kq * 4 : (kq + 1) * 4, FD : 2 * FD], wf[:])

            def x_chunk(ring, m):
                xf = xload.tile([P, KT, P], F32, name="xf", tag="xf")
                src = xt[m * P * KT * P : (m + 1) * P * KT * P].rearrange(
                    "(p f) -> p f", p=P
                )
                ring.dma_start(xf[:].rearrange("p a b -> p (a b)"), src)
                if m in ACT_X_TILES:
                    nc.scalar.sign(bx[:, :, m * P : (m + 1) * P], xf[:])
                else:
                    nc.vector.tensor_scalar(
                        bx[:, :, m * P : (m + 1) * P], xf[:], 0.0, 0.5,
                        op0=mybir.AluOpType.is_gt, op1=mybir.AluOpType.subtract,
                    )

            def wsl_chunk(ring, i):
                wsf = wsload.tile([P, 8, SL], F32, name="wsf", tag="wsf")
                src = wsl[i * P * 8 * SL : (i + 1) * P * 8 * SL].rearrange(
                    "(p f) -> p f", p=P
                )
                ring.dma_start(wsf[:].rearrange("p a b -> p (a b)"), src)
                nc.scalar.sign(bsl[:, i * 8 : (i + 1) * 8, :], wsf[:])

            # ring schedules --------------------------------------------------
            # scalar: w01 quads 0,1 | x0 x2 x4 x6 | outs-even (in unit loop)
            # sync:   w01 quads 2,3 | x1 | wsl | bounce | x3 x5 x7 | outs-odd
            # gpsimd: w23 halves, bounce slot is mid-queue; engine then issues
            #         the blocking AG, then agload descs fire on its ring.
            x_chunk(nc.scalar, 0)
            x_chunk(nc.sync, 1)
            w01_chunk(nc.scalar, 0)
            w01_chunk(nc.sync, 2)
            w01_chunk(nc.scalar, 1)
            w01_chunk(nc.sync, 3)
            w23_chunk(nc.gpsimd, 0)
            w23_chunk(nc.gpsimd, 1)
            wsl_chunk(nc.sync, 0)
            wsl_chunk(nc.sync, 1)
            nc.gpsimd.dma_start(
                wsg_in.ap().rearrange("(p f) -> p f", p=P),
                bsl[:].rearrange("p a b -> p (a b)"),
            )
            w23_chunk(nc.gpsimd, 2)
            w23_chunk(nc.gpsimd, 3)
            x_chunk(nc.scalar, 2)
            x_chunk(nc.sync, 3)
            x_chunk(nc.scalar, 4)
            x_chunk(nc.sync, 5)
            x_chunk(nc.scalar, 6)
            x_chunk(nc.sync, 7)

            nc.gpsimd.collective_compute(
                "AllGather",
                mybir.AluOpType.bypass,
                replica_groups=[list(range(N_CORES))],
                ins=[wsg_in.ap()],
                outs=[wsg_out.ap()],
            )
            # AG slot loads (gpsimd ring tail, gated on the AG): slots 4..7
            SLB = P * KT * SL
            for s in range(4, 8):
                src = wsg_out.ap()[s * SLB : (s + 1) * SLB].rearrange(
                    "(p f) -> p f", p=P
                )
                nc.gpsimd.dma_start(
                    bwr[:, s - 4, :, :].rearrange("p a b -> p (a b)"), src
                )

            # -- PE units ----------------------------------------------------
            # one FD-256 DoubleRow accumulation group per unit, in its own
            # PSUM bank (allocated [P, FD] so banks are never shared).
            ob8 = None
            for ui, (s, m) in enumerate(UNIT_ORDER):
                ps = ppool.tile([P, FD], F32, name="ps", tag="ps")
                if s < 4:
                    rhs = lambda kc: bwl[:, 2 * kc : 2 * kc + 2, s * SL : (s + 1) * SL]
                else:
                    rhs = lambda kc: bwr[:, s - 4, 2 * kc : 2 * kc + 2, :]
                for kc in range(KT // 2):
                    nc.tensor.matmul(
                        ps[:, 0:SL],
                        bx[:, 2 * kc : 2 * kc + 2, m * P : (m + 1) * P],
                        rhs(kc),
                        start=(kc == 0),
                        stop=(kc == KT // 2 - 1),
                        perf_mode=mybir.MatmulPerfMode.DoubleRow,
                    )
                if ui % 8 == 0:
                    ob8 = outp.tile([P, 8, SL], F16, name="ob", tag="ob")
                acol = 1 if m in ACT_X_TILES else 0
                nc.vector.tensor_scalar_mul(
                    ob8[:, ui % 8, :], ps[:, 0:SL], alpha_t[:, acol : acol + 1]
                )
                if ui % 8 == 7:
                    ring = nc.scalar if (ui // 8) % 2 == 0 else nc.sync
                    ring.dma_start(
                        out[ui // 8], ob8[:].rearrange("p a b -> p (a b)")
                    )

    nc.compile()
    return nc


def _pack_common(weight):
    WT4 = np.ascontiguousarray(weight.T).reshape(KT, P, OUTF)
    w01 = np.concatenate(
        [
            WT4[kq * 4 : (kq + 1) * 4, :, 0:FD].transpose(1, 0, 2).ravel()
            for kq in range(4)
        ]
    )
    w23 = np.concatenate(
        [
            WT4[kq * 4 : (kq + 1) * 4, :, FD : 2 * FD].transpose(1, 0, 2).ravel()
            for kq in range(4)
        ]
    )
    wsls = []
    for c in range(N_CORES):
        cols = slice(c * SL, (c + 1) * SL)
        wsls.append(
            np.ascontiguousarray(
                np.concatenate(
                    [
                        WT4[i * 8 : (i + 1) * 8, :, cols].transpose(1, 0, 2).ravel()
                        for i in range(2)
                    ]
                )
            )
        )
    return np.ascontiguousarray(w01), np.ascontiguousarray(w23), wsls


def _pack_x_shard(xs):
    xT4 = np.ascontiguousarray(xs.T).reshape(KT, P, TPC)
    return np.ascontiguousarray(
        np.concatenate(
            [xT4[:, :, m * P : (m + 1) * P].transpose(1, 0, 2).ravel() for m in range(MT)]
        )
    )


def kernel(x, weight, alpha):
    global _compiled, LAST_RESULT
    if _compiled is None:
        _compiled = _build()
    nc = _compiled

    x = np.asarray(x, dtype=np.float32)
    weight = np.asarray(weight, dtype=np.float32)
    alpha = np.asarray(alpha, dtype=np.float32)

    w01, w23, wsls = _pack_common(weight)
    a = float(alpha.reshape(-1)[0])
    alv = np.empty((P, 2), dtype=np.float32)
    alv[:, 0] = 2.0 * a
    alv[:, 1] = a
    in_maps = []
    for c in range(N_CORES):
        xs = _pack_x_shard(x[c * TPC : (c + 1) * TPC, :])
        in_maps.append(
            {"xt": xs, "w01": w01, "w23": w23, "wsl": wsls[c], "alpha": alv}
        )

    LAST_RESULT = run_bass_kernel_spmd(nc, in_maps, list(range(N_CORES)))
    full = np.empty((NTOK, OUTF), dtype=np.float32)
    for c in range(N_CORES):
        o = LAST_RESULT.results[c]["out"].astype(np.float32)  # [8, P, 8*SL]
        o = o.reshape(8, P, 8, SL)
        for ui, (s, m) in enumerate(UNIT_ORDER):
            rows = slice(c * TPC + m * P, c * TPC + (m + 1) * P)
            cols = slice(s * SL, (s + 1) * SL)
            full[rows, cols] = o[ui // 8, :, ui % 8, :]
    return full


# revision 36
# speedup vs baseline: 1.8118x; 1.4395x over previous
"""BinaryLinear kernel for Trainium2 (8 NeuronCores, SPMD).

Computes  out = sign(x) @ sign(W)^T * alpha  for
x: [8192, 2048] f32, W: [2048, 2048] f32, alpha: [1] f32.

Strategy: data-parallel over the token dim (8 shards of 1024 tokens);
W replicated. Host side packs inputs into flat per-chunk streams so
every DMA is a single fully-contiguous transfer with 4-8 KB runs per
SBUF partition, in exact consumption order. On device: sign() both
operands into resident fp8(E4M3) SBUF buffers (+-1 exact; accumulation
of <=2048 +-1 terms is exact in fp32 PSUM), then DoubleRow fp8 matmuls
(2 k-tiles per MM), scale by alpha on PSUM drain (DVE/ACT
alternating), write out per m-pair (contiguous staging layout, host
re-merges).

Rings: the sync (HWDGE) ring carries all W chunks in strict
consumption order (n0 small chunks interleaved with x by k-progress,
then n1, n2, n3 quads); the scalar (HWDGE) ring carries alpha, all x
chunks, then the output writes (gated by drains).
"""

import numpy as np

import concourse.bass as bass
import concourse.tile as tile
from concourse import bacc, mybir
from concourse.bass_utils import run_bass_kernel_spmd

N_CORES = 8
NTOK = 8192
INF = 2048
OUTF = 2048
TPC = NTOK // N_CORES  # tokens per core (1024)
P = 128
KT = INF // P  # 16 contraction tiles
MT = TPC // P  # 8 token tiles per core
NTS = 512  # out_features per matmul (one PSUM bank)
NT = OUTF // NTS  # 4

F32 = mybir.dt.float32
FP8 = mybir.dt.float8e4  # E4M3; +-1.0 is exact
SIGN_DT = FP8
K_STEP = 2  # contraction tiles per matmul (2 = fp8 DoubleRow)

# W chunk schedule per n-slice: n0 in small chunks (fine-grained pacing
# while x streams, tiny first chunks to fill the pipeline), n1..n3 in
# k-quads (1 MiB chunks, 8 KB/partition runs).
W_CHUNKS = {0: [1, 1, 2, 2, 2, 4, 4], 1: [4] * 4, 2: [4] * 4, 3: [4] * 4}
X_CHUNKS = [1, 1, 2, 2, 2, 2, 2, 2, 2]

_compiled = None
LAST_RESULT = None  # BassKernelResults of the most recent run (for profiling)


def _build():
    nc = bacc.Bacc(
        "TRN2",
        target_bir_lowering=False,
        debug=False,
        num_devices=N_CORES,
    )
    xt = nc.dram_tensor("xt", [KT * P * TPC], F32, kind="ExternalInput").ap()
    wt = nc.dram_tensor("wt", [NT * KT * P * NTS], F32, kind="ExternalInput").ap()
    al = nc.dram_tensor("alpha", [P, 1], F32, kind="ExternalInput").ap()
    out = nc.dram_tensor(
        "out", [NT, MT // 2, P, 2 * NTS], F32, kind="ExternalOutput"
    ).ap()

    with tile.TileContext(nc) as tc:
        with (
            tc.tile_pool(name="res", bufs=1) as res,
            tc.tile_pool(name="wload", bufs=4) as wload,
            tc.tile_pool(name="xload", bufs=3) as xload,
            tc.tile_pool(name="psum", bufs=8, space="PSUM") as ppool,
            tc.tile_pool(name="outp", bufs=2) as outp,
        ):
            # Resident sign() buffers (fp8)
            bw = res.tile([P, KT, OUTF], SIGN_DT)  # 32 KB/partition
            bx = res.tile([P, KT, TPC], SIGN_DT)  # 16 KB/partition
            alpha_t = res.tile([P, 1], F32)

            perf_mode = mybir.MatmulPerfMode.DoubleRow if K_STEP == 2 else None

            def mm(ps_ap, m, n, k):
                nc.tensor.matmul(
                    ps_ap,
                    bx[:, k : k + K_STEP, m * P : (m + 1) * P],
                    bw[:, k : k + K_STEP, n * NTS : (n + 1) * NTS],
                    start=(k == 0),
                    stop=(k + K_STEP >= KT),
                    perf_mode=perf_mode,
                )

            w_off = [0]

            def load_sign_w_chunk(n, k0, sz, engine):
                wf = wload.tile([P, sz, NTS], F32, name="wf", tag="wf")
                src = wt[w_off[0] : w_off[0] + P * sz * NTS].rearrange(
                    "(p f) -> p f", p=P
                )
                engine.dma_start(wf[:].rearrange("p a b -> p (a b)"), src)
                w_off[0] += P * sz * NTS
                for j in range(sz):
                    nc.scalar.sign(bw[:, k0 + j, n * NTS : (n + 1) * NTS], wf[:, j, :])

            x_off = [0]

            def load_sign_x_chunk(k0, sz, engine):
                xf = xload.tile([P, sz, TPC], F32, name="xf", tag="xf")
                src = xt[x_off[0] : x_off[0] + P * sz * TPC].rearrange(
                    "(p f) -> p f", p=P
                )
                engine.dma_start(xf[:].rearrange("p a b -> p (a b)"), src)
                x_off[0] += P * sz * TPC
                for j in range(sz):
                    nc.vector.tensor_scalar(
                        bx[:, k0 + j, :], xf[:, j, :], 0.0, None,
                        op0=mybir.AluOpType.is_gt,
                    )
                    nc.vector.tensor_scalar(
                        bx[:, k0 + j, :], bx[:, k0 + j, :], 2.0, -1.0,
                        op0=mybir.AluOpType.mult, op1=mybir.AluOpType.add,
                    )

            # ---- load + sign phase (issue order == consumption order) ----
            # gpsimd ring: x chunks. sync ring: all W chunks, n0 first
            # (interleaved with x by k-progress), then n1, n2, n3.
            nc.scalar.dma_start(alpha_t[:], al)

            def next_w_ring():
                return nc.sync

            xi = wi = xk = wk = 0
            while xi < len(X_CHUNKS) or wi < len(W_CHUNKS[0]):
                if xi < len(X_CHUNKS) and (wi >= len(W_CHUNKS[0]) or xk <= wk):
                    load_sign_x_chunk(xk, X_CHUNKS[xi], nc.scalar)
                    xk += X_CHUNKS[xi]
                    xi += 1
                else:
                    load_sign_w_chunk(0, wk, W_CHUNKS[0][wi], next_w_ring())
                    wk += W_CHUNKS[0][wi]
                    wi += 1
            for n in (1, 2, 3):
                k0 = 0
                for sz in W_CHUNKS[n]:
                    load_sign_w_chunk(n, k0, sz, next_w_ring())
                    k0 += sz

            def drain(dst, ps, idx, last_pass):
                # DVE drains mid-kernel (ACT is busy signing); alternate
                # DVE/ACT in the last pass so the tail drains in parallel.
                if not last_pass or idx % 2 == 0:
                    nc.vector.tensor_scalar_mul(dst, ps, alpha_t[:])
                else:
                    nc.scalar.activation(
                        dst, ps, mybir.ActivationFunctionType.Copy,
                        scale=alpha_t[:],
                    )

            # ---- matmul phase ----
            for n in range(NT):
                obuf = outp.tile([P, MT, NTS], F32)
                if n < 2:
                    # streaming passes: k-middle / m-inner
                    pss = [
                        ppool.tile([P, NTS], F32, name="ps", tag="ps")
                        for _ in range(MT)
                    ]
                    for k in range(0, KT, K_STEP):
                        for m in range(MT):
                            mm(pss[m][:], m, n, k)
                    for m in range(MT):
                        drain(obuf[:, m, :], pss[m][:], m, n == NT - 1)
                        if m % 2 == 1:
                            nc.scalar.dma_start(
                                out[n, m // 2],
                                obuf[:, m - 1 : m + 1, :].rearrange(
                                    "p a b -> p (a b)"
                                ),
                            )
                else:
                    # resident passes: m-outer / k-inner
                    for m in range(MT):
                        ps = ppool.tile([P, NTS], F32, name="ps", tag="ps")
                        for k in range(0, KT, K_STEP):
                            mm(ps[:], m, n, k)
                        drain(obuf[:, m, :], ps[:], m, n == NT - 1)
                        if m % 2 == 1:
                            nc.scalar.dma_start(
                                out[n, m // 2],
                                obuf[:, m - 1 : m + 1, :].rearrange(
                                    "p a b -> p (a b)"
                                ),
                            )

    nc.compile()
    return nc


def _pack_w(weight):
    # WT4[k, p, n, c] = W^T[(k*128+p), n*512+c]
    wt4 = weight.T.reshape(KT, P, NT, NTS)
    parts = []
    for n in range(NT):
        k0 = 0
        for sz in W_CHUNKS[n]:
            parts.append(
                wt4[k0 : k0 + sz, :, n, :].transpose(1, 0, 2).ravel()
            )
            k0 += sz
    return np.ascontiguousarray(np.concatenate(parts))


def _pack_x_shard(xs):
    # xs: [TPC, INF] -> xT4[k, p, t]
    xt4 = xs.T.reshape(KT, P, TPC)
    parts = []
    k0 = 0
    for sz in X_CHUNKS:
        parts.append(xt4[k0 : k0 + sz].transpose(1, 0, 2).ravel())
        k0 += sz
    return np.ascontiguousarray(np.concatenate(parts))


def kernel(x, weight, alpha):
    global _compiled, LAST_RESULT
    if _compiled is None:
        _compiled = _build()
    nc = _compiled

    x = np.asarray(x, dtype=np.float32)
    weight = np.asarray(weight, dtype=np.float32)
    alpha = np.asarray(alpha, dtype=np.float32)

    wt = _pack_w(weight)
    alv = np.full((P, 1), alpha.reshape(-1)[0], dtype=np.float32)
    in_maps = []
    for c in range(N_CORES):
        xs = _pack_x_shard(x[c * TPC : (c + 1) * TPC, :])
        in_maps.append({"xt": xs, "wt": wt, "alpha": alv})

    LAST_RESULT = run_bass_kernel_spmd(nc, in_maps, list(range(N_CORES)))
    outs = []
    for c in range(N_CORES):
        o = LAST_RESULT.results[c]["out"]  # [NT, MT//2, P, 2*NTS]
        o = o.reshape(NT, MT // 2, P, 2, NTS)
        # -> [MT//2, 2, P, NT, NTS] -> [TPC, OUTF]
        outs.append(o.transpose(1, 3, 2, 0, 4).reshape(TPC, OUTF))
    return np.concatenate(outs, axis=0)



# revision 39
# speedup vs baseline: 1.8468x; 1.0193x over previous
"""BinaryLinear kernel for Trainium2 (8 NeuronCores, SPMD).

Computes  out = sign(x) @ sign(W)^T * alpha  for
x: [8192, 2048] f32, W: [2048, 2048] f32, alpha: [1] f32.

Strategy: data-parallel over the token dim (8 shards of 1024 tokens);
W replicated. Host side packs inputs into flat per-chunk streams so
every DMA is a single fully-contiguous transfer with 4-8 KB runs per
SBUF partition, in exact consumption order. On device: sign() both
operands into resident fp8(E4M3) SBUF buffers (+-1 exact; accumulation
of <=2048 +-1 terms is exact in fp32 PSUM), then DoubleRow fp8 matmuls
(2 k-tiles per MM), scale by alpha on PSUM drain (DVE/ACT
alternating), write out per m-pair (contiguous staging layout, host
re-merges).

Rings: the sync (HWDGE) ring carries all W chunks in strict
consumption order (n0 small chunks interleaved with x by k-progress,
then n1, n2, n3 quads); the scalar (HWDGE) ring carries alpha, all x
chunks, then the output writes (gated by drains).
"""

import numpy as np

import concourse.bass as bass
import concourse.tile as tile
from concourse import bacc, mybir
from concourse.bass_utils import run_bass_kernel_spmd

N_CORES = 8
NTOK = 8192
INF = 2048
OUTF = 2048
TPC = NTOK // N_CORES  # tokens per core (1024)
P = 128
KT = INF // P  # 16 contraction tiles
MT = TPC // P  # 8 token tiles per core
NTS = 512  # out_features per matmul (one PSUM bank)
NT = OUTF // NTS  # 4

F32 = mybir.dt.float32
F16 = mybir.dt.float16  # out dtype; sums are integers <= 2048, exact in f16
FP8 = mybir.dt.float8e4  # E4M3; +-1.0 is exact
SIGN_DT = FP8
K_STEP = 2  # contraction tiles per matmul (2 = fp8 DoubleRow)

# W chunk schedule per n-slice: n0 in small chunks (fine-grained pacing
# while x streams, tiny first chunks to fill the pipeline), n1..n3 in
# k-quads (1 MiB chunks, 8 KB/partition runs).
W_CHUNKS = {0: [1, 1, 2, 2, 2, 4, 4], 1: [4] * 4, 2: [4] * 4, 3: [4] * 4}
X_CHUNKS = [1, 1, 2, 2, 2, 2, 2, 2, 2]

_compiled = None
LAST_RESULT = None  # BassKernelResults of the most recent run (for profiling)


def _build():
    nc = bacc.Bacc(
        "TRN2",
        target_bir_lowering=False,
        debug=False,
        num_devices=N_CORES,
    )
    xt = nc.dram_tensor("xt", [KT * P * TPC], F32, kind="ExternalInput").ap()
    wt = nc.dram_tensor("wt", [NT * KT * P * NTS], F32, kind="ExternalInput").ap()
    al = nc.dram_tensor("alpha", [P, 1], F32, kind="ExternalInput").ap()
    out = nc.dram_tensor(
        "out", [NT, MT // 2, P, 2 * NTS], F16, kind="ExternalOutput"
    ).ap()

    with tile.TileContext(nc) as tc:
        with (
            tc.tile_pool(name="res", bufs=1) as res,
            tc.tile_pool(name="wload", bufs=4) as wload,
            tc.tile_pool(name="xload", bufs=3) as xload,
            tc.tile_pool(name="psum", bufs=8, space="PSUM") as ppool,
            tc.tile_pool(name="outp", bufs=2) as outp,
        ):
            # Resident sign() buffers (fp8)
            bw = res.tile([P, KT, OUTF], SIGN_DT)  # 32 KB/partition
            bx = res.tile([P, KT, TPC], SIGN_DT)  # 16 KB/partition
            alpha_t = res.tile([P, 1], F32)

            perf_mode = mybir.MatmulPerfMode.DoubleRow if K_STEP == 2 else None

            def mm(ps_ap, m, n, k):
                nc.tensor.matmul(
                    ps_ap,
                    bx[:, k : k + K_STEP, m * P : (m + 1) * P],
                    bw[:, k : k + K_STEP, n * NTS : (n + 1) * NTS],
                    start=(k == 0),
                    stop=(k + K_STEP >= KT),
                    perf_mode=perf_mode,
                )

            w_off = [0]

            def load_sign_w_chunk(n, k0, sz, engine):
                wf = wload.tile([P, sz, NTS], F32, name="wf", tag="wf")
                src = wt[w_off[0] : w_off[0] + P * sz * NTS].rearrange(
                    "(p f) -> p f", p=P
                )
                engine.dma_start(wf[:].rearrange("p a b -> p (a b)"), src)
                w_off[0] += P * sz * NTS
                for j in range(sz):
                    nc.scalar.sign(bw[:, k0 + j, n * NTS : (n + 1) * NTS], wf[:, j, :])

            x_off = [0]

            def load_sign_x_chunk(k0, sz, engine):
                xf = xload.tile([P, sz, TPC], F32, name="xf", tag="xf")
                src = xt[x_off[0] : x_off[0] + P * sz * TPC].rearrange(
                    "(p f) -> p f", p=P
                )
                engine.dma_start(xf[:].rearrange("p a b -> p (a b)"), src)
                x_off[0] += P * sz * TPC
                for j in range(sz):
                    nc.vector.tensor_scalar(
                        bx[:, k0 + j, :], xf[:, j, :], 0.0, None,
                        op0=mybir.AluOpType.is_gt,
                    )
                    nc.vector.tensor_scalar(
                        bx[:, k0 + j, :], bx[:, k0 + j, :], 2.0, -1.0,
                        op0=mybir.AluOpType.mult, op1=mybir.AluOpType.add,
                    )

            # ---- load + sign phase (issue order == consumption order) ----
            # gpsimd ring: x chunks. sync ring: all W chunks, n0 first
            # (interleaved with x by k-progress), then n1, n2, n3.
            nc.scalar.dma_start(alpha_t[:], al)

            def next_w_ring():
                return nc.sync

            xi = wi = xk = wk = 0
            while xi < len(X_CHUNKS) or wi < len(W_CHUNKS[0]):
                if xi < len(X_CHUNKS) and (wi >= len(W_CHUNKS[0]) or xk <= wk):
                    load_sign_x_chunk(xk, X_CHUNKS[xi], nc.scalar)
                    xk += X_CHUNKS[xi]
                    xi += 1
                else:
                    load_sign_w_chunk(0, wk, W_CHUNKS[0][wi], next_w_ring())
                    wk += W_CHUNKS[0][wi]
                    wi += 1
            for n in (1, 2, 3):
                k0 = 0
                for sz in W_CHUNKS[n]:
                    load_sign_w_chunk(n, k0, sz, next_w_ring())
                    k0 += sz

            def drain(dst, ps, idx, last_pass):
                # DVE drains mid-kernel (ACT is busy signing); alternate
                # DVE/ACT in the last pass so the tail drains in parallel.
                if not last_pass or idx % 2 == 0:
                    nc.vector.tensor_scalar_mul(dst, ps, alpha_t[:])
                else:
                    nc.scalar.activation(
                        dst, ps, mybir.ActivationFunctionType.Copy,
                        scale=alpha_t[:],
                    )

            # ---- matmul phase ----
            for n in range(NT):
                obuf = outp.tile([P, MT, NTS], F16)
                if n < 2:
                    # streaming passes: k-middle / m-inner
                    pss = [
                        ppool.tile([P, NTS], F32, name="ps", tag="ps")
                        for _ in range(MT)
                    ]
                    for k in range(0, KT, K_STEP):
                        for m in range(MT):
                            mm(pss[m][:], m, n, k)
                    for m in range(MT):
                        drain(obuf[:, m, :], pss[m][:], m, n == NT - 1)
                        if m % 2 == 1:
                            nc.scalar.dma_start(
                                out[n, m // 2],
                                obuf[:, m - 1 : m + 1, :].rearrange(
                                    "p a b -> p (a b)"
                                ),
                            )
                else:
                    # resident passes: m-outer / k-inner
                    for m in range(MT):
                        ps = ppool.tile([P, NTS], F32, name="ps", tag="ps")
                        for k in range(0, KT, K_STEP):
                            mm(ps[:], m, n, k)
                        drain(obuf[:, m, :], ps[:], m, n == NT - 1)
                        if m % 2 == 1:
                            nc.scalar.dma_start(
                                out[n, m // 2],
                                obuf[:, m - 1 : m + 1, :].rearrange(
                                    "p a b -> p (a b)"
                                ),
                            )

    nc.compile()
    return nc


def _pack_w(weight):
    # WT4[k, p, n, c] = W^T[(k*128+p), n*512+c]
    wt4 = weight.T.reshape(KT, P, NT, NTS)
    parts = []
    for n in range(NT):
        k0 = 0
        for sz in W_CHUNKS[n]:
            parts.append(
                wt4[k0 : k0 + sz, :, n, :].transpose(1, 0, 2).ravel()
            )
            k0 += sz
    return np.ascontiguousarray(np.concatenate(parts))


def _pack_x_shard(xs):
    # xs: [TPC, INF] -> xT4[k, p, t]
    xt4 = xs.T.reshape(KT, P, TPC)
    parts = []
    k0 = 0
    for sz in X_CHUNKS:
        parts.append(xt4[k0 : k0 + sz].transpose(1, 0, 2).ravel())
        k0 += sz
    return np.ascontiguousarray(np.concatenate(parts))


def kernel(x, weight, alpha):
    global _compiled, LAST_RESULT
    if _compiled is None:
        _compiled = _build()
    nc = _compiled

    x = np.asarray(x, dtype=np.float32)
    weight = np.asarray(weight, dtype=np.float32)
    alpha = np.asarray(alpha, dtype=np.float32)

    wt = _pack_w(weight)
    alv = np.full((P, 1), alpha.reshape(-1)[0], dtype=np.float32)
    in_maps = []
    for c in range(N_CORES):
        xs = _pack_x_shard(x[c * TPC : (c + 1) * TPC, :])
        in_maps.append({"xt": xs, "wt": wt, "alpha": alv})

    LAST_RESULT = run_bass_kernel_spmd(nc, in_maps, list(range(N_CORES)))
    outs = []
    for c in range(N_CORES):
        o = LAST_RESULT.results[c]["out"].astype(np.float32)
        o = o.reshape(NT, MT // 2, P, 2, NTS)
        # -> [MT//2, 2, P, NT, NTS] -> [TPC, OUTF]
        outs.append(o.transpose(1, 3, 2, 0, 4).reshape(TPC, OUTF))
    return np.concatenate(outs, axis=0)

